# revision 2
# baseline (speedup 1.0000x reference)
import hashlib
import os
import sys
import tempfile

import numpy as np

sys.path.insert(0, "/opt/trn_rl_repo")

import concourse.bass as bass
import concourse.tile as tile
from concourse import mybir
from concourse.masks import make_identity

F32 = mybir.dt.float32
BF16 = mybir.dt.bfloat16
I32 = mybir.dt.int32
AF = mybir.ActivationFunctionType
OP = mybir.AluOpType

H = 8
BIG = 100.0
N_GRAPHS = 512
NEFF_CACHE_DIR = "/tmp/gat_bass_neff_cache"

GEOM = dict(
    n_cores=8,
    W=98,
    S=18,
    layers=[(18, 64), (64, 128), (128, 128), (128, 128)],
)


def _ap_ndims(arg):
    ap = getattr(arg, "ap", None)
    if ap is None:
        bap = getattr(arg, "bass_ap", None)
        ap = getattr(bap, "ap", None) if bap is not None else None
    try:
        return len(ap) if ap is not None else 0
    except TypeError:
        return 0


def split_excess_waits(nc, max_waits=1):
    import copy
    proto = nc.vector.nop().ins
    n_split = 0
    for bb in nc.main_func.blocks:
        out = []
        for ins in bb.instructions:
            si = getattr(ins, "sync_info", None)
            tname = type(ins).__name__
            if (si is not None and si.on_wait and len(si.on_wait) > max_waits
                    and "EventSemaphore" not in tname
                    and "NoOp" not in tname):
                for wt in list(si.on_wait):
                    nop = copy.deepcopy(proto)
                    n_split += 1
                    nop.name = f"NOPW-{n_split}"
                    nop.engine = ins.engine
                    nop.sync_info = mybir.SyncInfo(on_wait=[wt], on_update=[])
                    out.append(nop)
                si.on_wait = []
            out.append(ins)
        bb.instructions[:] = out
    return n_split


def build_gat_nc(geom):
    n_cores = geom["n_cores"]
    W = geom["W"]
    S = geom["S"]
    layers = geom["layers"]
    NS = W * 128
    NW = n_cores * W
    N_pad = NW * 128
    nl = len(layers)

    nc = bass.Bass()

    C1 = layers[0][0]
    xT0 = nc.declare_dram_parameter("xT0", [C1, NS], F32, isOutput=False)
    src_idx = nc.declare_dram_parameter("src_idx", [128, W * S], I32, isOutput=False)
    dst_idx = nc.declare_dram_parameter("dst_idx", [128, W * S], I32, isOutput=False)
    dstloc = nc.declare_dram_parameter("dstloc", [128, W * S], F32, isOutput=False)
    bnd = nc.declare_dram_parameter("bnd", [128, W], F32, isOutput=False)
    Wc = []
    bb_ = []
    for li, (ci, co) in enumerate(layers):
        Wc.append(nc.declare_dram_parameter(f"Wc{li}", [ci, co + 16], F32, isOutput=False))
        bb_.append(nc.declare_dram_parameter(f"bb{li}", [128, co], F32, isOutput=False))
    pool_out = nc.declare_dram_parameter("pool_out", [128, W * 4], F32, isOutput=True)

    rg = [list(range(n_cores))]

    with tile.TileContext(nc) as tc:
        with (tc.tile_pool(name="const", bufs=1) as const,
              tc.tile_pool(name="dram", bufs=1, space="DRAM") as dram,
              tc.tile_pool(name="a_sb", bufs=3) as a_sb,
              tc.tile_pool(name="a_ps", bufs=4, space="PSUM") as a_ps,
              tc.tile_pool(name="b_sb", bufs=3) as b_sb,
              tc.tile_pool(name="b_ps", bufs=2, space="PSUM") as b_ps):
            ident = const.tile([128, 128], F32)
            make_identity(nc, ident[:])
            iota_f = const.tile([128, 128], F32)
            nc.gpsimd.iota(iota_f[:], pattern=[[1, 128]], base=0,
                           channel_multiplier=0,
                           allow_small_or_imprecise_dtypes=True)
            src_sb = const.tile([128, W * S], I32)
            nc.sync.dma_start(out=src_sb[:], in_=src_idx[:])
            dst_sb = const.tile([128, W * S], I32)
            nc.sync.dma_start(out=dst_sb[:], in_=dst_idx[:])
            dl_sb = const.tile([128, W * S], F32)
            nc.sync.dma_start(out=dl_sb[:], in_=dstloc[:])
            bnd_sb = const.tile([128, W], F32)
            nc.sync.dma_start(out=bnd_sb[:], in_=bnd[:])
            Wc_sb = []
            bb_sb = []
            for li, (ci, co) in enumerate(layers):
                w_t = const.tile([ci, co + 16], F32, name=f"wc_sb{li}")
                nc.sync.dma_start(out=w_t[:], in_=Wc[li][:])
                Wc_sb.append(w_t)
                b_t = const.tile([128, co], F32, name=f"bb_sb{li}")
                nc.sync.dma_start(out=b_t[:], in_=bb_[li][:])
                bb_sb.append(b_t)
            pool_acc = const.tile([128, W * 4], F32)

            xTf = []
            for li, (ci, co) in enumerate(layers):
                xTf.append(dram.tile([n_cores * ci, NS], F32, addr_space="Shared",
                                     name=f"xTf{li}"))
            xT0b = dram.tile([C1, NS], F32, name="xT0b")
            nc.sync.dma_start(out=xT0b[:], in_=xT0[:])
            nc.gpsimd.collective_compute(
                "AllGather", OP.bypass, replica_groups=rg,
                ins=[xT0b[:]], outs=[xTf[0][:]],
            )

            for li, (ci, co) in enumerate(layers):
                F = co + 8
                cpl = co // H
                tbl = dram.tile([N_pad, F], F32, name=f"tbl{li}")
                ald = dram.tile([N_pad, H], F32, name=f"ald{li}")
                xTs = dram.tile([co, NS], F32, name=f"xTs{li}")

                for wg2 in range(0, NW, 2):
                    lhsT = a_sb.tile([ci, 256], F32, tag="a_lhsT")
                    r0 = (wg2 // W) * ci
                    c0 = (wg2 % W) * 128
                    nc.sync.dma_start(out=lhsT[:], in_=xTf[li][r0:r0 + ci, c0:c0 + 256])
                    for j in range(2):
                        wg = wg2 + j
                        ps = a_ps.tile([128, co + 16], F32, tag="a_ps")
                        nc.tensor.matmul(ps[:], lhsT=lhsT[:, j * 128:(j + 1) * 128],
                                         rhs=Wc_sb[li][:], start=True, stop=True)
                        osb = a_sb.tile([128, co + 16], F32, tag="a_osb")
                        nc.vector.tensor_copy(osb[:], ps[:])
                        n0 = wg * 128
                        nc.sync.dma_start(out=tbl[n0:n0 + 128, :], in_=osb[:, 0:F])
                        nc.sync.dma_start(out=ald[n0:n0 + 128, :], in_=osb[:, F:F + 8])

                for w in range(W):
                    g = b_sb.tile([128, S * F], F32, tag="g")
                    nc.gpsimd.indirect_dma_start(
                        out=g[:], out_offset=None, in_=tbl[:],
                        in_offset=bass.IndirectOffsetOnAxis(
                            ap=src_sb[:, w * S:(w + 1) * S], axis=0),
                    )
                    ad = b_sb.tile([128, S * H], F32, tag="ad")
                    nc.gpsimd.indirect_dma_start(
                        out=ad[:], out_offset=None, in_=ald[:],
                        in_offset=bass.IndirectOffsetOnAxis(
                            ap=dst_sb[:, w * S:(w + 1) * S], axis=0),
                    )
                    mask = b_sb.tile([128, S * 128], BF16, tag="mask")
                    nc.vector.tensor_tensor(
                        out=mask[:].rearrange("p (t j) -> p t j", t=S),
                        in0=iota_f[:].unsqueeze(1).broadcast_to([128, S, 128]),
                        in1=dl_sb[:, w * S:(w + 1) * S].unsqueeze(-1)
                            .broadcast_to([128, S, 128]),
                        op=OP.is_equal,
                    )
                    e = b_sb.tile([128, S * H], F32, tag="e")
                    g3 = g[:].rearrange("p (t f) -> p t f", t=S)
                    nc.vector.tensor_tensor(
                        out=e[:].rearrange("p (t h) -> p t h", t=S),
                        in0=g3[:, :, co:co + H],
                        in1=ad[:].rearrange("p (t h) -> p t h", t=S),
                        op=OP.add,
                    )
                    lk = b_sb.tile([128, S * H], F32, tag="lk")
                    nc.vector.tensor_scalar(out=lk[:], in0=e[:], scalar1=0.2,
                                            scalar2=None, op0=OP.mult)
                    nc.vector.tensor_tensor(out=lk[:], in0=e[:], in1=lk[:], op=OP.max)
                    ex = b_sb.tile([128, S * H], F32, tag="ex")
                    nc.scalar.activation(out=ex[:], in_=lk[:], func=AF.Exp)
                    rhs = b_sb.tile([128, S * F], BF16, tag="rhs")
                    rhs3 = rhs[:].rearrange("p (t f) -> p t f", t=S)
                    nc.vector.tensor_tensor(
                        out=rhs3[:, :, 0:co].rearrange("p t (h c) -> p t h c", h=H),
                        in0=g3[:, :, 0:co].rearrange("p t (h c) -> p t h c", h=H),
                        in1=ex[:].rearrange("p (t h) -> p t h", t=S)
                            .unsqueeze(-1).broadcast_to([128, S, H, cpl]),
                        op=OP.mult,
                    )
                    nc.scalar.activation(
                        out=rhs3[:, :, co:co + H],
                        in_=ex[:].rearrange("p (t h) -> p t h", t=S),
                        func=AF.Copy)
                    acc = b_ps.tile([128, F], F32, tag="acc")
                    for t in range(S):
                        nc.tensor.matmul(acc[:],
                                         lhsT=mask[:, t * 128:(t + 1) * 128],
                                         rhs=rhs[:, t * F:(t + 1) * F],
                                         start=(t == 0), stop=(t == S - 1))
                    den = b_sb.tile([128, H], F32, tag="den")
                    nc.vector.tensor_scalar(out=den[:], in0=acc[:, co:co + H],
                                            scalar1=1e-12, scalar2=None, op0=OP.max)
                    rden = b_sb.tile([128, H], F32, tag="rden")
                    nc.vector.reciprocal(rden[:], den[:])
                    xz = b_sb.tile([128, co], F32, tag="xz")
                    nc.vector.tensor_tensor(
                        out=xz[:].rearrange("p (h c) -> p h c", h=H),
                        in0=acc[:, 0:co].rearrange("p (h c) -> p h c", h=H),
                        in1=rden[:].unsqueeze(-1).broadcast_to([128, H, cpl]),
                        op=OP.mult,
                    )
                    zb = b_sb.tile([128, co], F32, tag="zb")
                    nc.vector.tensor_tensor(out=zb[:], in0=xz[:], in1=bb_sb[li][:],
                                            op=OP.add)
                    zm = b_sb.tile([128, co], F32, tag="zm")
                    nc.vector.tensor_scalar(out=zm[:], in0=zb[:], scalar1=0.0,
                                            scalar2=None, op0=OP.min)
                    em = b_sb.tile([128, co], F32, tag="em")
                    nc.scalar.activation(out=em[:], in_=zm[:], func=AF.Exp)
                    nc.vector.tensor_scalar(out=zm[:], in0=em[:], scalar1=1.0,
                                            scalar2=None, op0=OP.subtract)
                    xf = b_sb.tile([128, co], F32, tag="xf")
                    nc.vector.tensor_tensor(out=xf[:], in0=zb[:], in1=zm[:], op=OP.max)
                    pT = b_ps.tile([co, 128], F32, tag="pT")
                    nc.tensor.transpose(out=pT[:], in_=xf[:], identity=ident[:])
                    xts_sb = b_sb.tile([co, 128], F32, tag="xts_sb")
                    nc.vector.tensor_copy(xts_sb[:], pT[:])
                    nc.sync.dma_start(out=xTs[:, w * 128:(w + 1) * 128], in_=xts_sb[:])

                    if li == nl - 1:
                        m0 = b_sb.tile([128, 128], F32, tag="m0")
                        nc.vector.tensor_scalar(out=m0[:], in0=iota_f[:],
                                                scalar1=bnd_sb[:, w:w + 1],
                                                scalar2=None, op0=OP.is_lt)
                        m1 = b_sb.tile([128, 128], F32, tag="m1")
                        nc.vector.tensor_scalar(out=m1[:], in0=iota_f[:],
                                                scalar1=bnd_sb[:, w:w + 1],
                                                scalar2=None, op0=OP.is_ge)
                        xb = b_sb.tile([128, 128], F32, tag="xb")
                        nc.vector.tensor_scalar(out=xb[:], in0=xts_sb[:], scalar1=BIG,
                                                scalar2=None, op0=OP.add)
                        tmp = b_sb.tile([128, 128], F32, tag="ptmp")
                        for pc, mm in enumerate((m0, m1)):
                            nc.vector.tensor_tensor(out=tmp[:], in0=xts_sb[:],
                                                    in1=mm[:], op=OP.mult)
                            nc.vector.tensor_reduce(
                                out=pool_acc[:, w * 4 + pc:w * 4 + pc + 1],
                                in_=tmp[:], axis=mybir.AxisListType.X, op=OP.add)
                            nc.vector.tensor_tensor(out=tmp[:], in0=xb[:],
                                                    in1=mm[:], op=OP.mult)
                            nc.vector.tensor_reduce(
                                out=pool_acc[:, w * 4 + 2 + pc:w * 4 + 3 + pc],
                                in_=tmp[:], axis=mybir.AxisListType.X, op=OP.max)

                if li < nl - 1:
                    nc.gpsimd.collective_compute(
                        "AllGather", OP.bypass, replica_groups=rg,
                        ins=[xTs[:]], outs=[xTf[li + 1][:]],
                    )

            nc.sync.dma_start(out=pool_out[:], in_=pool_acc[:])

    split_excess_waits(nc)
    return nc


def host_prep(x, edge_index, batch, Ws, As, Ad, Bs, geom):
    n_cores, W, S = geom["n_cores"], geom["W"], geom["S"]
    layers = geom["layers"]
    NS = W * 128
    NW = n_cores * W
    N_pad = NW * 128
    n = x.shape[0]
    E = edge_index.shape[1]

    src = np.empty(E + n, np.int32)
    dst = np.empty(E + n, np.int32)
    src[:E] = edge_index[0]
    dst[:E] = edge_index[1]
    src[E:] = np.arange(n, dtype=np.int32)
    dst[E:] = src[E:]

    order = np.argsort(dst, kind="stable")
    src_s = src[order]
    dst_s = dst[order]

    counts = np.bincount(dst, minlength=N_pad)
    starts = np.zeros(N_pad + 1, np.int64)
    np.cumsum(counts, out=starts[1:])
    wstart = starts[::128][:NW]
    wcnt = starts[np.arange(1, NW + 1) * 128] - wstart
    assert wcnt.max() <= S * 128, f"window overflow: {wcnt.max()} > {S * 128}"

    ET = E + n
    win_of_edge = dst_s >> 7
    j_in_win = np.arange(ET, dtype=np.int64) - wstart[win_of_edge]
    slot = win_of_edge * (S * 128) + j_in_win

    A_src = np.zeros(NW * S * 128, np.int32)
    A_dst = np.zeros(NW * S * 128, np.int32)
    A_dl = np.full(NW * S * 128, 999.0, np.float32)
    A_src[slot] = src_s
    A_dst[slot] = dst_s
    A_dl[slot] = (dst_s - (win_of_edge << 7)).astype(np.float32)

    def percore(Aflat, dt):
        A = Aflat.reshape(n_cores, W, S, 128)
        return [np.ascontiguousarray(
            A[c].transpose(2, 0, 1).reshape(128, W * S)).astype(dt)
            for c in range(n_cores)]

    src_pc = percore(A_src, np.int32)
    dst_pc = percore(A_dst, np.int32)
    dl_pc = percore(A_dl, np.float32)

    G = N_GRAPHS
    gcnt = np.bincount(batch, minlength=G)
    gstarts = np.zeros(G + 1, np.int64)
    np.cumsum(gcnt, out=gstarts[1:])
    bnd_full = np.full(NW, 128.0, np.float32)
    bpts = np.unique(np.concatenate([gstarts[1:G], [n]]))
    bpts = bpts[(bpts > 0) & (bpts < N_pad)]
    wb = bpts >> 7
    off = bpts & 127
    interior = off > 0
    wbi, offi = wb[interior], off[interior]
    assert len(np.unique(wbi)) == len(wbi), "more than one boundary in a window"
    bnd_full[wbi] = offi.astype(np.float32)
    bnd_pc = [np.ascontiguousarray(np.broadcast_to(
        bnd_full[c * W:(c + 1) * W], (128, W))).astype(np.float32)
        for c in range(n_cores)]

    C1 = layers[0][0]
    xp = np.zeros((N_pad, C1), np.float32)
    xp[:n] = x
    xT = np.ascontiguousarray(xp.T)
    xT_pc = [np.ascontiguousarray(xT[:, c * NS:(c + 1) * NS]) for c in range(n_cores)]

    Wc_l, bb_l = [], []
    for li, (ci, co) in enumerate(layers):
        c = co // H
        As_bd = np.zeros((co, H), np.float32)
        Ad_bd = np.zeros((co, H), np.float32)
        for h in range(H):
            As_bd[h * c:(h + 1) * c, h] = As[li][h]
            Ad_bd[h * c:(h + 1) * c, h] = Ad[li][h]
        Wl = Ws[li].astype(np.float32)
        Wc_l.append(np.ascontiguousarray(
            np.concatenate([Wl, Wl @ As_bd, Wl @ Ad_bd], axis=1)))
        bb_l.append(np.ascontiguousarray(
            np.broadcast_to(Bs[li].astype(np.float32), (128, co))))

    in_maps = []
    for cidx in range(n_cores):
        m = dict(xT0=xT_pc[cidx], src_idx=src_pc[cidx], dst_idx=dst_pc[cidx],
                 dstloc=dl_pc[cidx], bnd=bnd_pc[cidx])
        for li in range(len(layers)):
            m[f"Wc{li}"] = Wc_l[li]
            m[f"bb{li}"] = bb_l[li]
        in_maps.append(m)

    post = dict(bnd_full=bnd_full, batch=np.asarray(batch), n=n, G=G,
                gcnt=gcnt, NW=NW, W=W, n_cores=n_cores)
    return in_maps, post


def host_post(pool_outs, post, fcW, fcb):
    n, G, W = post["n"], post["G"], post["W"]
    batch = post["batch"]
    bnd_full = post["bnd_full"]
    co = pool_outs[0].shape[0]
    sums = np.zeros((G, co), np.float64)
    maxs = np.full((G, co), -np.inf, np.float64)
    n_windows_real = (n + 127) // 128
    for c in range(post["n_cores"]):
        po = pool_outs[c].reshape(co, W, 4)
        for wl in range(W):
            wg = c * W + wl
            if wg >= n_windows_real:
                break
            node0 = wg * 128
            b = int(bnd_full[wg])
            g0 = int(batch[node0])
            sums[g0] += po[:, wl, 0]
            maxs[g0] = np.maximum(maxs[g0], po[:, wl, 2] - BIG)
            if b < 128 and node0 + b < n:
                g1 = int(batch[node0 + b])
                sums[g1] += po[:, wl, 1]
                maxs[g1] = np.maximum(maxs[g1], po[:, wl, 3] - BIG)
    cnt = post["gcnt"].astype(np.float64)
    mean = sums / np.maximum(cnt, 1.0)[:, None]
    maxs[cnt == 0] = 0.0
    maxs[np.isinf(maxs)] = 0.0
    feat = np.concatenate([mean, maxs], axis=1).astype(np.float32)
    z = feat @ fcW + fcb
    z = z - z.max(axis=1, keepdims=True)
    z = z - np.log(np.exp(z).sum(axis=1, keepdims=True))
    return z.astype(np.float32)


def _install_cached_cc_hook():
    from concourse import bass2jax
    if getattr(bass2jax, "_gat_cc_cache_installed", False):
        return
    orig_hook = bass2jax.neuronx_cc_hook

    def cached_hook(code, code_format, platform_version, file_prefix):
        cb = bytes(code)
        if b"bass_exec" not in cb:
            return orig_hook(code, code_format, platform_version, file_prefix)
        key = hashlib.sha256(cb).hexdigest()
        path = os.path.join(NEFF_CACHE_DIR, key + ".bin")
        if os.path.exists(path):
            with open(path, "rb") as f:
                return 0, f.read()
        ret = orig_hook(code, code_format, platform_version, file_prefix)
        try:
            rc, blob = ret
            if rc == 0 and isinstance(blob, (bytes, bytearray)):
                os.makedirs(NEFF_CACHE_DIR, exist_ok=True)
                fd, tmp = tempfile.mkstemp(dir=NEFF_CACHE_DIR)
                with os.fdopen(fd, "wb") as f:
                    f.write(blob)
                os.replace(tmp, path)
        except Exception:
            pass
        return ret

    bass2jax.neuronx_cc_hook = cached_hook
    bass2jax._gat_cc_cache_installed = True


def kernel(x, edge_index, batch,
           W1, a1s, a1d, b1, W2, a2s, a2d, b2,
           W3, a3s, a3d, b3, W4, a4s, a4d, b4, fcW, fcb):
    x = np.asarray(x, np.float32)
    edge_index = np.asarray(edge_index)
    batch = np.asarray(batch)
    Ws = [np.asarray(W1), np.asarray(W2), np.asarray(W3), np.asarray(W4)]
    As = [np.asarray(a1s), np.asarray(a2s), np.asarray(a3s), np.asarray(a4s)]
    Ad = [np.asarray(a1d), np.asarray(a2d), np.asarray(a3d), np.asarray(a4d)]
    Bs = [np.asarray(b1), np.asarray(b2), np.asarray(b3), np.asarray(b4)]

    _install_cached_cc_hook()
    in_maps, post = host_prep(x, edge_index, batch, Ws, As, Ad, Bs, GEOM)
    nc = build_gat_nc(GEOM)

    from concourse.bass_utils import run_bass_kernel_spmd
    res = run_bass_kernel_spmd(nc, in_maps, list(range(GEOM["n_cores"])))
    pool_outs = [res.results[c]["pool_out"] for c in range(GEOM["n_cores"])]
    return host_post(pool_outs, post, np.asarray(fcW, np.float32),
                     np.asarray(fcb, np.float32))
